# revision 24
# baseline (speedup 1.0000x reference)
"""Trainium2 Bass kernel for DifferentiableRBFSVMModel forward.

Math (reference):
    dist[n,s] = max(x_sq[n] + xi_sq[s] - 2*cross[n,s], 0)
    K = exp(-g*dist);  res = sigmoid(K @ (alphas*yis) + intercept)   -> [1, N]

Factorization used on device (clamp dropped: dist >= 0 up to fp eps, and
exp(-g*eps) == 1 to ~1e-7):
    K[n,s] = exp(-g*x_sq[n]) * exp(2g*cross[n,s] - g*xi_sq[s])
    res[n] = sigmoid(A[n] * sum_s E[s,n]*w[s] + c),  A = exp(-g*x_sq), w = alphas*yis
    sigmoid(z) = 0.5 + 0.5*tanh(0.5*z)   (tanh is in the same ACT table set as exp)

Sharding: data-parallel over N across 8 cores (x columns of xT), everything
else replicated.  Per core (NS = 2048 rows of x):
    mm1 (PE):  crossT psum tiles [128s x 1024n] = xisT_tile^T @ xT  (fp16, fp32 acc)
    ACT:       E = exp(2g*psum + bias_s)  (bias = -g*xi_sq per-partition), fp16
    mm2 (PE):  po[1, n] += w_tile^T @ E   (M=1, accumulated across all 64 s-tiles,
               emitted 2 stages behind so ACT never stalls PE)
    tail:      z = A*po, sigmoid via tanh, DMA out.

DMA: xisT is split into 8 column-chunks per d-half; chunks 2+ are gated on a
marker read of an earlier stage's psum so their DMA doesn't compete with the
critical first 2 MB during the prologue.
"""

import numpy as np

N, D, S, NCORES = 16384, 256, 8192, 8
NS = N // NCORES          # 2048 rows of x per core
TS = S // 128             # 64 s-tiles
GAMMA = 0.00390625        # 1/256
XCH = 8                   # xisT column chunks per d-half (1024 cols each)
LAG = 2                   # mm2 stages behind mm1


def _build_bass():
    import concourse.bacc as bacc
    import concourse.mybir as mybir
    import concourse.tile as tile

    f32 = mybir.dt.float32
    f16 = mybir.dt.float16
    AF = mybir.ActivationFunctionType

    nc = bacc.Bacc("TRN2", target_bir_lowering=False, debug=False)

    xT_d = nc.dram_tensor("xT", [2, 128, NS], f16, kind="ExternalInput")
    xisT_d = nc.dram_tensor("xisT", [2, 128, S], f16, kind="ExternalInput")
    biasS_d = nc.dram_tensor("biasS", [128, TS], f32, kind="ExternalInput")
    w_d = nc.dram_tensor("w", [128, TS], f16, kind="ExternalInput")
    A_d = nc.dram_tensor("A", [1, NS], f32, kind="ExternalInput")
    ch_d = nc.dram_tensor("chalf", [1, 1], f32, kind="ExternalInput")
    out_d = nc.dram_tensor("out", [1, NS], f32, kind="ExternalOutput")

    cw = S // XCH  # 1024

    with tile.TileContext(nc) as tc:
        with (
            tc.tile_pool(name="big", bufs=1) as big,
            tc.tile_pool(name="epool", bufs=6) as epool,
            tc.tile_pool(name="spool", bufs=2) as spool,
            tc.tile_pool(name="psumc", bufs=3, space="PSUM") as psumc,
            tc.tile_pool(name="psumo", bufs=1, space="PSUM") as psumo,
        ):
            # xt split into h-halves so block 0's first mms wait on 0.5 MB,
            # not the full critical set.
            xt = []
            for d in range(2):
                t = big.tile([128, NS], f16, tag=f"xt{d}", name=f"xt{d}")
                nc.sync.dma_start(out=t[:, 0:1024], in_=xT_d.ap()[d][:, 0:1024])
                xt.append(t)
            biasS = big.tile([128, TS], f32, tag="biasS", name="biasS")
            nc.sync.dma_start(out=biasS, in_=biasS_d.ap())
            wsb = big.tile([128, TS], f16, tag="w", name="wsb")
            nc.sync.dma_start(out=wsb, in_=w_d.ap())
            Asb = big.tile([1, NS], f32, tag="A", name="Asb")
            nc.sync.dma_start(out=Asb, in_=A_d.ap())
            chs = big.tile([1, 1], f32, tag="chalf", name="chs")
            nc.sync.dma_start(out=chs, in_=ch_d.ap())

            # Warmup ACTs: walrus attaches the activation-table-load waits
            # here (few Tile waits) instead of the first pipeline exp.
            wsrc = big.tile([1, 1], f32, tag="wsrc", name="wsrc")
            nc.vector.memset(wsrc, 0.0)
            wdst = big.tile([1, 1], f32, tag="wdst", name="wdst")
            nc.scalar.activation(wdst, wsrc, AF.Tanh)
            nc.scalar.activation(wdst, wsrc, AF.Exp)

            # xisT chunk tiles; chunks 0-1 DMA'd up front, the rest gated.
            xis = {}
            for c in range(XCH):
                for d in range(2):
                    xis[(d, c)] = big.tile(
                        [128, cw], f16, tag=f"xis{d}_{c}", name=f"xis{d}_{c}"
                    )
            for d in range(2):
                nc.sync.dma_start(
                    out=xis[(d, 0)][:, 0:512], in_=xisT_d.ap()[d][:, 0:512]
                )
            for d in range(2):
                nc.sync.dma_start(
                    out=xt[d][:, 1024:NS], in_=xT_d.ap()[d][:, 1024:NS]
                )
            for d in range(2):
                nc.sync.dma_start(
                    out=xis[(d, 0)][:, 512:cw], in_=xisT_d.ap()[d][:, 512:cw]
                )

            gate = big.tile([1, XCH], f32, tag="gate", name="gate")

            # po: one PSUM bank; n-chunk c accumulates at partition 32c
            # (M=1 matmul output col-group-packed via out base_partition).
            po = psumo.tile([128, 512], f32, tag="po", name="po")

            def emit_mm2(t, es):
                for h, e in enumerate(es):
                    for q in range(2):
                        cch = h * 2 + q
                        nc.tensor.matmul(
                            po[32 * cch : 32 * cch + 1, 0:512],
                            wsb[:, t : t + 1],
                            e[:, q * 512 : (q + 1) * 512],
                            start=(t == 0),
                            stop=(t == TS - 1),
                            skip_group_check=True,
                            tile_position=(0, 32 * cch),
                        )

            pending = []
            for t in range(TS):
                c, o = t // XCH, (t % XCH) * 128
                pc = [
                    psumc.tile([128, 1024], f32, tag="pc", name=f"pc_{t}_{h}")
                    for h in range(2)
                ]
                es = []
                for h in range(2):
                    for d in range(2):
                        lhs = xis[(d, c)][:, o : o + 128]
                        for q in range(2):
                            lo = h * 1024 + q * 512
                            nc.tensor.matmul(
                                pc[h][:, q * 512 : (q + 1) * 512],
                                lhs,
                                xt[d][:, lo : lo + 512],
                                start=(d == 0),
                                stop=(d == 1),
                            )
                    e = epool.tile([128, 1024], f16, tag="E", name=f"E_{t}_{h}")
                    nc.scalar.activation(
                        e, pc[h], AF.Exp, bias=biasS[:, t : t + 1], scale=2.0 * GAMMA
                    )
                    es.append(e)
                # Gate chunk c+2's DMA on this stage's psum: the marker copy
                # waits for mm1(t), and the DMA (WAW on the chunk tile) waits
                # for the marker — so the chunk loads ~8 stages before use
                # without competing with the prologue-critical DMAs.
                if t % 4 == 0 and t // 4 + 1 < XCH:
                    cn = t // 4 + 1
                    nc.vector.tensor_copy(gate[0:1, cn : cn + 1], pc[0][0:1, 0:1])
                    for d in range(2):
                        nc.vector.tensor_copy(
                            xis[(d, cn)][0:1, 0:1], gate[0:1, cn : cn + 1]
                        )
                        nc.sync.dma_start(
                            out=xis[(d, cn)],
                            in_=xisT_d.ap()[d][:, cn * cw : (cn + 1) * cw],
                        )
                pending.append((t, es))
                if len(pending) > LAG:
                    emit_mm2(*pending.pop(0))
            for args in pending:
                emit_mm2(*args)

            # Tail, pipelined in 4 chunks so DVE-mul, ACT-tanh, DVE-affine
            # and the out-DMA overlap instead of running serially on [1, NS].
            TC = 4
            tw = NS // TC
            ALU = mybir.AluOpType
            for i in range(TC):
                sl = slice(i * tw, (i + 1) * tw)
                z = spool.tile([1, tw], f32, tag="z", name=f"z_{i}")
                nc.vector.tensor_mul(z, po[32 * i : 32 * i + 1, :], Asb[0:1, sl])
                th = spool.tile([1, tw], f32, tag="th", name=f"th_{i}")
                nc.scalar.activation(th, z, AF.Tanh, bias=chs[0:1, 0:1], scale=0.5)
                ob = spool.tile([1, tw], f32, tag="ob", name=f"ob_{i}")
                nc.vector.tensor_scalar(
                    out=ob, in0=th, scalar1=0.5, scalar2=0.5, op0=ALU.mult, op1=ALU.add
                )
                nc.sync.dma_start(out=out_d.ap()[:, sl], in_=ob)

    nc.compile()
    return nc


_NC_CACHE = None


def _get_nc():
    global _NC_CACHE
    if _NC_CACHE is None:
        _NC_CACHE = _build_bass()
    return _NC_CACHE


def _prep_inputs(x, alphas, xis, yis, intercept):
    x = np.asarray(x, np.float32)
    xis = np.asarray(xis, np.float32)
    alphas = np.asarray(alphas, np.float32)
    yis = np.asarray(yis, np.float32)
    intercept = np.asarray(intercept, np.float32)

    xT = np.ascontiguousarray(x.T).reshape(2, 128, N).astype(np.float16)
    xisT = np.ascontiguousarray(xis.T).reshape(2, 128, S).astype(np.float16)
    xi_sq = np.sum(xis * xis, axis=1)                      # [S]
    x_sq = np.sum(x * x, axis=1)                           # [N]
    biasS = np.ascontiguousarray(
        (-GAMMA * xi_sq).reshape(TS, 128).T
    ).astype(np.float32)                                   # [128, TS]
    w = np.ascontiguousarray(
        (alphas * yis).reshape(TS, 128).T
    ).astype(np.float16)                                   # [128, TS]
    A = np.exp(-GAMMA * x_sq).astype(np.float32)           # [N]
    chalf = (0.5 * intercept[0]) * np.ones((1, 1), np.float32)

    in_maps = []
    for c in range(NCORES):
        sl = slice(c * NS, (c + 1) * NS)
        in_maps.append(
            {
                "xT": np.ascontiguousarray(xT[:, :, sl]),
                "xisT": xisT,
                "biasS": biasS,
                "w": w,
                "A": np.ascontiguousarray(A[sl]).reshape(1, NS),
                "chalf": chalf,
            }
        )
    return in_maps


def kernel(x, alphas, xis, yis, intercept, _trace=False):
    from concourse import bass_utils

    nc = _get_nc()
    in_maps = _prep_inputs(x, alphas, xis, yis, intercept)
    res = bass_utils.run_bass_kernel_spmd(
        nc, in_maps, core_ids=list(range(NCORES)), trace=_trace
    )
    out = np.concatenate([res.results[c]["out"] for c in range(NCORES)], axis=1)
    if _trace:
        return out.astype(np.float32), res
    return out.astype(np.float32)


# revision 25
# speedup vs baseline: 1.0013x; 1.0013x over previous
"""Trainium2 Bass kernel for DifferentiableRBFSVMModel forward.

Math (reference):
    dist[n,s] = max(x_sq[n] + xi_sq[s] - 2*cross[n,s], 0)
    K = exp(-g*dist);  res = sigmoid(K @ (alphas*yis) + intercept)   -> [1, N]

Factorization used on device (clamp dropped: dist >= 0 up to fp eps, and
exp(-g*eps) == 1 to ~1e-7):
    K[n,s] = exp(-g*x_sq[n]) * exp(2g*cross[n,s] - g*xi_sq[s])
    res[n] = sigmoid(A[n] * sum_s E[s,n]*w[s] + c),  A = exp(-g*x_sq), w = alphas*yis
    sigmoid(z) = 0.5 + 0.5*tanh(0.5*z)   (tanh is in the same ACT table set as exp)

Sharding: data-parallel over N across 8 cores (x columns of xT), everything
else replicated.  Per core (NS = 2048 rows of x):
    mm1 (PE):  crossT psum tiles [128s x 1024n] = xisT_tile^T @ xT  (fp16, fp32 acc)
    ACT:       E = exp(2g*psum + bias_s)  (bias = -g*xi_sq per-partition), fp16
    mm2 (PE):  po[1, n] += w_tile^T @ E   (M=1, accumulated across all 64 s-tiles,
               emitted 2 stages behind so ACT never stalls PE)
    tail:      z = A*po, sigmoid via tanh, DMA out.

DMA: xisT is split into 8 column-chunks per d-half; chunks 2+ are gated on a
marker read of an earlier stage's psum so their DMA doesn't compete with the
critical first 2 MB during the prologue.
"""

import numpy as np

N, D, S, NCORES = 16384, 256, 8192, 8
NS = N // NCORES          # 2048 rows of x per core
TS = S // 128             # 64 s-tiles
GAMMA = 0.00390625        # 1/256
XCH = 8                   # xisT column chunks per d-half (1024 cols each)
LAG = 2                   # mm2 stages behind mm1


def _build_bass():
    import concourse.bacc as bacc
    import concourse.mybir as mybir
    import concourse.tile as tile

    f32 = mybir.dt.float32
    f16 = mybir.dt.float16
    AF = mybir.ActivationFunctionType

    nc = bacc.Bacc("TRN2", target_bir_lowering=False, debug=False)

    xT_d = nc.dram_tensor("xT", [2, 128, NS], f16, kind="ExternalInput")
    xisT_d = nc.dram_tensor("xisT", [2, 128, S], f16, kind="ExternalInput")
    biasS_d = nc.dram_tensor("biasS", [128, TS], f32, kind="ExternalInput")
    w_d = nc.dram_tensor("w", [128, TS], f16, kind="ExternalInput")
    A_d = nc.dram_tensor("A", [1, NS], f32, kind="ExternalInput")
    ch_d = nc.dram_tensor("chalf", [1, 1], f32, kind="ExternalInput")
    out_d = nc.dram_tensor("out", [1, NS], f32, kind="ExternalOutput")

    cw = S // XCH  # 1024

    with tile.TileContext(nc) as tc:
        with (
            tc.tile_pool(name="big", bufs=1) as big,
            tc.tile_pool(name="epool", bufs=6) as epool,
            tc.tile_pool(name="spool", bufs=2) as spool,
            tc.tile_pool(name="psumc", bufs=3, space="PSUM") as psumc,
            tc.tile_pool(name="psumo", bufs=1, space="PSUM") as psumo,
        ):
            # xt as separate h-half tiles: block 0's first mms then wait on
            # only xt-h0 + the first xis half-chunk (~0.75 MB critical).
            xth = {}
            for h in range(2):
                for d in range(2):
                    t = big.tile([128, 1024], f16, tag=f"xt{d}_{h}", name=f"xt{d}_{h}")
                    if h == 0:
                        nc.sync.dma_start(
                            out=t, in_=xT_d.ap()[d][:, h * 1024 : (h + 1) * 1024]
                        )
                    xth[(d, h)] = t
            biasS = big.tile([128, TS], f32, tag="biasS", name="biasS")
            nc.sync.dma_start(out=biasS, in_=biasS_d.ap())
            wsb = big.tile([128, TS], f16, tag="w", name="wsb")
            nc.sync.dma_start(out=wsb, in_=w_d.ap())
            Asb = big.tile([1, NS], f32, tag="A", name="Asb")
            nc.sync.dma_start(out=Asb, in_=A_d.ap())
            chs = big.tile([1, 1], f32, tag="chalf", name="chs")
            nc.sync.dma_start(out=chs, in_=ch_d.ap())

            # Warmup ACTs: walrus attaches the activation-table-load waits
            # here (few Tile waits) instead of the first pipeline exp.
            wsrc = big.tile([1, 1], f32, tag="wsrc", name="wsrc")
            nc.vector.memset(wsrc, 0.0)
            wdst = big.tile([1, 1], f32, tag="wdst", name="wdst")
            nc.scalar.activation(wdst, wsrc, AF.Tanh)
            nc.scalar.activation(wdst, wsrc, AF.Exp)

            # xisT chunk tiles; chunks 0-1 DMA'd up front, the rest gated.
            xis = {}
            for c in range(XCH):
                for d in range(2):
                    xis[(d, c)] = big.tile(
                        [128, cw], f16, tag=f"xis{d}_{c}", name=f"xis{d}_{c}"
                    )
            xisa = [
                big.tile([128, 512], f16, tag=f"xisa{d}", name=f"xisa{d}")
                for d in range(2)
            ]
            for d in range(2):
                nc.sync.dma_start(
                    out=xisa[d], in_=xisT_d.ap()[d][:, 0:512]
                )
            for d in range(2):
                nc.sync.dma_start(
                    out=xth[(d, 1)], in_=xT_d.ap()[d][:, 1024:2048]
                )
            for d in range(2):
                nc.sync.dma_start(
                    out=xis[(d, 0)][:, 512:cw], in_=xisT_d.ap()[d][:, 512:cw]
                )

            gate = big.tile([1, XCH], f32, tag="gate", name="gate")

            # po: one PSUM bank; n-chunk c accumulates at partition 32c
            # (M=1 matmul output col-group-packed via out base_partition).
            po = psumo.tile([128, 512], f32, tag="po", name="po")

            def emit_mm2(t, es):
                for h, e in enumerate(es):
                    for q in range(2):
                        cch = h * 2 + q
                        nc.tensor.matmul(
                            po[32 * cch : 32 * cch + 1, 0:512],
                            wsb[:, t : t + 1],
                            e[:, q * 512 : (q + 1) * 512],
                            start=(t == 0),
                            stop=(t == TS - 1),
                            skip_group_check=True,
                            tile_position=(0, 32 * cch),
                        )

            pending = []
            for t in range(TS):
                c, o = t // XCH, (t % XCH) * 128
                pc = [
                    psumc.tile([128, 1024], f32, tag="pc", name=f"pc_{t}_{h}")
                    for h in range(2)
                ]
                es = []
                for h in range(2):
                    for d in range(2):
                        if t < 4:
                            lhs = xisa[d][:, o : o + 128]
                        else:
                            lhs = xis[(d, c)][:, o : o + 128]
                        for q in range(2):
                            nc.tensor.matmul(
                                pc[h][:, q * 512 : (q + 1) * 512],
                                lhs,
                                xth[(d, h)][:, q * 512 : (q + 1) * 512],
                                start=(d == 0),
                                stop=(d == 1),
                            )
                    e = epool.tile([128, 1024], f16, tag="E", name=f"E_{t}_{h}")
                    nc.scalar.activation(
                        e, pc[h], AF.Exp, bias=biasS[:, t : t + 1], scale=2.0 * GAMMA
                    )
                    es.append(e)
                # Gate chunk c+2's DMA on this stage's psum: the marker copy
                # waits for mm1(t), and the DMA (WAW on the chunk tile) waits
                # for the marker — so the chunk loads ~8 stages before use
                # without competing with the prologue-critical DMAs.
                if t % 4 == 0 and t // 4 + 1 < XCH:
                    cn = t // 4 + 1
                    nc.vector.tensor_copy(gate[0:1, cn : cn + 1], pc[0][0:1, 0:1])
                    for d in range(2):
                        nc.vector.tensor_copy(
                            xis[(d, cn)][0:1, 0:1], gate[0:1, cn : cn + 1]
                        )
                        nc.sync.dma_start(
                            out=xis[(d, cn)],
                            in_=xisT_d.ap()[d][:, cn * cw : (cn + 1) * cw],
                        )
                pending.append((t, es))
                if len(pending) > LAG:
                    emit_mm2(*pending.pop(0))
            for args in pending:
                emit_mm2(*args)

            # Tail, pipelined in 4 chunks so DVE-mul, ACT-tanh, DVE-affine
            # and the out-DMA overlap instead of running serially on [1, NS].
            TC = 4
            tw = NS // TC
            ALU = mybir.AluOpType
            for i in range(TC):
                sl = slice(i * tw, (i + 1) * tw)
                z = spool.tile([1, tw], f32, tag="z", name=f"z_{i}")
                nc.vector.tensor_mul(z, po[32 * i : 32 * i + 1, :], Asb[0:1, sl])
                th = spool.tile([1, tw], f32, tag="th", name=f"th_{i}")
                nc.scalar.activation(th, z, AF.Tanh, bias=chs[0:1, 0:1], scale=0.5)
                ob = spool.tile([1, tw], f32, tag="ob", name=f"ob_{i}")
                nc.vector.tensor_scalar(
                    out=ob, in0=th, scalar1=0.5, scalar2=0.5, op0=ALU.mult, op1=ALU.add
                )
                nc.sync.dma_start(out=out_d.ap()[:, sl], in_=ob)

    nc.compile()
    return nc


_NC_CACHE = None


def _get_nc():
    global _NC_CACHE
    if _NC_CACHE is None:
        _NC_CACHE = _build_bass()
    return _NC_CACHE


def _prep_inputs(x, alphas, xis, yis, intercept):
    x = np.asarray(x, np.float32)
    xis = np.asarray(xis, np.float32)
    alphas = np.asarray(alphas, np.float32)
    yis = np.asarray(yis, np.float32)
    intercept = np.asarray(intercept, np.float32)

    xT = np.ascontiguousarray(x.T).reshape(2, 128, N).astype(np.float16)
    xisT = np.ascontiguousarray(xis.T).reshape(2, 128, S).astype(np.float16)
    xi_sq = np.sum(xis * xis, axis=1)                      # [S]
    x_sq = np.sum(x * x, axis=1)                           # [N]
    biasS = np.ascontiguousarray(
        (-GAMMA * xi_sq).reshape(TS, 128).T
    ).astype(np.float32)                                   # [128, TS]
    w = np.ascontiguousarray(
        (alphas * yis).reshape(TS, 128).T
    ).astype(np.float16)                                   # [128, TS]
    A = np.exp(-GAMMA * x_sq).astype(np.float32)           # [N]
    chalf = (0.5 * intercept[0]) * np.ones((1, 1), np.float32)

    in_maps = []
    for c in range(NCORES):
        sl = slice(c * NS, (c + 1) * NS)
        in_maps.append(
            {
                "xT": np.ascontiguousarray(xT[:, :, sl]),
                "xisT": xisT,
                "biasS": biasS,
                "w": w,
                "A": np.ascontiguousarray(A[sl]).reshape(1, NS),
                "chalf": chalf,
            }
        )
    return in_maps


def kernel(x, alphas, xis, yis, intercept, _trace=False):
    from concourse import bass_utils

    nc = _get_nc()
    in_maps = _prep_inputs(x, alphas, xis, yis, intercept)
    res = bass_utils.run_bass_kernel_spmd(
        nc, in_maps, core_ids=list(range(NCORES)), trace=_trace
    )
    out = np.concatenate([res.results[c]["out"] for c in range(NCORES)], axis=1)
    if _trace:
        return out.astype(np.float32), res
    return out.astype(np.float32)
